# revision 16
# baseline (speedup 1.0000x reference)
"""Multi-head self-attention with additive position bias, data-parallel across
8 TRN2 NeuronCores (one batch element per core).

Per core (batch b), everything is computed in a transposed layout so that no
on-device transposes are needed:
  - host supplies xT = x[b].T (fp16) and epos[h] = exp(pos[h].T / sqrt(D)) (fp16)
  - qT/kT    = W_{q,k}.T @ xT                    [cols, N]   (PE, fp16)
  - v        = xT.T @ W_v                        [N, cols]   (PE, fp16)
  - scoresT  = kT_h(m-tile).T @ qT_h             [m, n]      (PE, head-pairs
               packed into row groups 0-63 / 64-127 of the systolic array)
  - estT     = exp(scoresT/sqrt(D)) * eposT      (ACT exp + DVE mul; the
               additive bias becomes a multiplicative factor after exp)
  - outT_h   = v_h.T @ estT  (accumulated over m-tiles; two heads col-packed)
  - sums_h   = 1.T @ estT    (softmax denominators, M=1 matmuls col-packed)
  - normalize: outT_h * (1/sums_h) broadcast via a tiny ones-matmul
  - out      = attnT.T @ W_proj                  [N, C] fp32
"""

import numpy as np

N_CORES = 8
N = 1024
C = 768
H = 12
D = 64
HP = H // 2  # head pairs
SCALE = 0.125  # 1/sqrt(D)

# ---------------------------------------------------------------------------
# walrus in this toolchain rejects instructions carrying more than one sync
# wait ("Too many sync wait commands").  Tile's semaphore pass can attach
# several (esp. the kernel-tail drain).  Spread surplus waits across InstNoOp
# instructions inserted immediately before the oversubscribed instruction in
# the same basic block / engine stream — semantically identical, since the
# engine sequencer performs the waits in stream order.
# ---------------------------------------------------------------------------


def _apply_tile_patch():
    from concourse import mybir
    from concourse.tile import TileContext
    from concourse.vector_clock import ScopedClock

    def _patched_drain_and_barrier(self, tick_clock, wait_clock):
        nc = self.nc
        drain_inst = nc.sync.drain()
        wait_clock.add_sem_waits(
            drain_inst.ins, ScopedClock({None: tick_clock.global_clock})
        )
        nc.all_engine_barrier()
        assert self.sems is not None
        popped = nc._tile_sem_poison_stack.pop()
        assert popped is self._sem_poison
        nc.clear_and_free_semaphores(list(self.sems.allocated().values()))
        nc.all_engine_barrier()

    TileContext._drain_and_barrier = _patched_drain_and_barrier


def _split_excess_waits(nc, max_waits=1):
    from concourse import mybir

    n_split = 0
    for f in nc.m.functions:
        for blk in f.blocks:
            insts = blk.instructions
            new_list = []
            changed = False
            for inst in insts:
                si = inst.sync_info
                waits = list(si.on_wait) if (si is not None and si.on_wait) else []
                if len(waits) > max_waits:
                    extra = waits[: len(waits) - max_waits]
                    keep = waits[len(waits) - max_waits :]
                    for i in range(0, len(extra), max_waits):
                        nop = mybir.InstNoOp(
                            name=nc.get_next_instruction_name(),
                            engine=inst.engine,
                            ins=[],
                            outs=[],
                            sync_info=mybir.SyncInfo(
                                on_wait=extra[i : i + max_waits], on_update=[]
                            ),
                        )
                        nc.register_instruction(nop, overwrite=True)
                        new_list.append(nop)
                        n_split += 1
                    inst.sync_info = mybir.SyncInfo(
                        on_wait=keep,
                        on_update=list(si.on_update) if si.on_update else [],
                    )
                    changed = True
                new_list.append(inst)
            if changed:
                blk.instructions = new_list
    return n_split


def build(has_bias):
    import concourse.bass as bass
    import concourse.mybir as mybir
    from concourse.tile import TileContext

    _apply_tile_patch()

    FP16 = mybir.dt.float16
    F32 = mybir.dt.float32
    EXP = mybir.ActivationFunctionType.Exp

    nc = bass.Bass()
    xt_ext = nc.declare_dram_parameter("xt", [C, N], FP16, isOutput=False)
    wqkv_ext = nc.declare_dram_parameter("wqkv", [C, 3 * C], FP16, isOutput=False)
    wproj_ext = nc.declare_dram_parameter("wproj", [C, C], FP16, isOutput=False)
    epos_ext = nc.declare_dram_parameter("epos", [H, N, N], FP16, isOutput=False)
    if has_bias:
        bqkv_ext = nc.declare_dram_parameter("bqkv", [1, 3 * C], FP16, isOutput=False)
        bproj_ext = nc.declare_dram_parameter("bproj", [1, C], FP16, isOutput=False)
    out_ext = nc.declare_dram_parameter("out", [N, C], F32, isOutput=True)

    KT = C // 128  # 6 contraction tiles
    NT = N // 128  # 8 n-tiles / m-tiles

    with TileContext(nc) as tc:
        with (
            tc.tile_pool(name="const", bufs=1) as const,
            tc.tile_pool(name="epp", bufs=4) as epp_pool,
            tc.tile_pool(name="est", bufs=11) as est_pool,
            tc.tile_pool(name="sgp", bufs=2) as sgp_pool,
            tc.tile_pool(name="invr", bufs=3) as invr_pool,
            tc.tile_pool(name="outsb", bufs=2) as outsb_pool,
            tc.tile_pool(name="ps", bufs=1, space="PSUM") as ps,
        ):
            XT = const.tile([128, KT, N], FP16)
            WQKV = const.tile([128, KT, 3 * C], FP16)
            WPROJ = const.tile([128, KT, C], FP16)
            xt_r = xt_ext.rearrange("(t p) n -> p t n", p=128)
            wqkv_r = wqkv_ext.rearrange("(t p) n -> p t n", p=128)
            # per-k-tile pieces, v columns first: the v-projection can start
            # as soon as the first k-tile of x and Wv has landed
            for kt in range(KT):
                nc.sync.dma_start(out=XT[:, kt, :], in_=xt_r[:, kt, :])
                nc.sync.dma_start(
                    out=WQKV[:, kt, 2 * C : 3 * C], in_=wqkv_r[:, kt, 2 * C : 3 * C]
                )
            nc.sync.dma_start(out=WQKV[:, :, 0 : 2 * C], in_=wqkv_r[:, :, 0 : 2 * C])
            nc.sync.dma_start(out=WPROJ[:], in_=wproj_ext.rearrange("(t p) n -> p t n", p=128))
            if has_bias:
                BQKV = const.tile([1, 3 * C], FP16)
                BPROJ = const.tile([1, C], FP16)
                ONESROW = const.tile([1, N], FP16)
                nc.sync.dma_start(out=BQKV[:], in_=bqkv_ext[:])
                nc.sync.dma_start(out=BPROJ[:], in_=bproj_ext[:])
                nc.vector.memset(ONESROW[:], 1.0)

            ONES128 = const.tile([128, 1], FP16)
            ONES1x64 = const.tile([1, 64], FP16)
            nc.vector.memset(ONES128[:], 1.0)
            nc.vector.memset(ONES1x64[:], 1.0)

            # warm up the PE clock (HAM) with throwaway matmuls while the
            # initial DMAs are still in flight
            WARM = const.tile([128, 512], FP16)
            nc.vector.memset(WARM[:], 0.0)
            pw = ps.tile([128, 512], F32, tag="bc")
            for i in range(24):
                nc.tensor.matmul(pw[:], WARM[:, 0:128], WARM[:], start=(i == 0), stop=(i == 23))

            # per pair hp: qT of heads (2hp, 2hp+1) at [0:N], kT at [N:2N]
            QKT = const.tile([128, HP, 2 * N], FP16)
            VN = const.tile([128, NT, C], FP16)
            ATTNT = const.tile([128, KT, N], FP16)

            # ---- V projection: v[n, vcol] = xT.T @ Wv (+ b_v) ----
            _vtags = ["sc", "oacc", "sums", "bc"]
            for nt in range(NT):
                for vs in range(2):
                    pv = ps.tile([128, 384], F32, tag=_vtags[(nt * 2 + vs) % 4])
                    dst = pv[:, 0:384]
                    for kt in range(KT):
                        nc.tensor.matmul(
                            dst,
                            XT[:, kt, nt * 128 : (nt + 1) * 128],
                            WQKV[:, kt, 2 * C + vs * 384 : 2 * C + (vs + 1) * 384],
                            start=(kt == 0),
                            stop=(kt == KT - 1 and not has_bias),
                        )
                    if has_bias:
                        nc.tensor.matmul(
                            dst,
                            ONESROW[0:1, nt * 128 : (nt + 1) * 128],
                            BQKV[0:1, 2 * C + vs * 384 : 2 * C + (vs + 1) * 384],
                            start=False,
                            stop=True,
                        )
                    nc.vector.tensor_copy(VN[:, nt, vs * 384 : (vs + 1) * 384], dst)

            # ---- head-pair loop, software-pipelined one pair deep:
            # pair hp:   scores -> exp -> est     (ACT-bound phase)
            # pair hp-1: attn.v + sums matmuls    (dense PE work, fills gaps)
            # pair hp+1: qT/kT projection chunks  (always-ready PE filler that
            #            keeps the HAM activity window busy -> PE stays warm)

            def qkt_chunk(pair, c):
                # chunk c of pair: c in 0..3 -> (q ns0, q ns1, k ns0, k ns1)
                ct = pair if c < 2 else HP + pair
                col0 = ct * 128
                ns = c % 2
                pqc = ps.tile([128, 512], F32, tag="bc")
                for kt in range(KT):
                    nc.tensor.matmul(
                        pqc[:],
                        WQKV[:, kt, col0 : col0 + 128],
                        XT[:, kt, ns * 512 : (ns + 1) * 512],
                        start=(kt == 0),
                        stop=(kt == KT - 1 and not has_bias),
                    )
                if has_bias:
                    nc.tensor.matmul(
                        pqc[:],
                        BQKV[0:1, col0 : col0 + 128],
                        ONESROW[0:1, ns * 512 : (ns + 1) * 512],
                        start=False,
                        stop=True,
                    )
                nc.vector.tensor_copy(QKT[:, pair, c * 512 : (c + 1) * 512], pqc[:])

            for c in range(4):
                qkt_chunk(0, c)

            prev = None  # (hp, [EST per mt])
            for hp in range(HP + 1):
                if hp < HP:
                    h0, h1 = 2 * hp, 2 * hp + 1
                cur = []
                OUTP = SMS = None
                ph0 = ph1 = None
                if prev is not None:
                    ph, pest = prev
                    ph0, ph1 = 2 * ph, 2 * ph + 1
                for mt in range(8):
                    # phase 2 of the previous pair first: its inputs are all
                    # ready, so the PE never stalls entering an iteration
                    if prev is not None:
                        pest_mt = prev[1][mt]
                        if mt == 0:
                            OUTP = ps.tile([128, 1024], F32, tag="oacc")
                            # one PSUM bank: rows 0/32 = h0 (ns0/ns1), 64/96 = h1
                            SMS = ps.tile([97, 512], F32, tag="sums")
                        first, last = (mt == 0), (mt == 7)
                        for ns in range(2):
                            nsl = slice(ns * 512, (ns + 1) * 512)
                            nsl1 = slice(N + ns * 512, N + (ns + 1) * 512)
                            nc.tensor.matmul(
                                OUTP[0:64, nsl],
                                VN[:, mt, ph0 * D : (ph0 + 1) * D],
                                pest_mt[:, nsl],
                                start=first, stop=last,
                            )
                            nc.tensor.matmul(
                                OUTP[64:128, nsl],
                                VN[:, mt, ph1 * D : (ph1 + 1) * D],
                                pest_mt[:, nsl1],
                                start=first, stop=last,
                                tile_position=(0, 64),
                            )
                        for ns in range(2):
                            nsl = slice(ns * 512, (ns + 1) * 512)
                            nsl1 = slice(N + ns * 512, N + (ns + 1) * 512)
                            r0, r1 = 32 * ns, 64 + 32 * ns
                            nc.tensor.matmul(
                                SMS[r0 : r0 + 1, :], ONES128[:], pest_mt[:, nsl],
                                start=first, stop=last,
                                tile_position=(0, r0),
                            )
                            nc.tensor.matmul(
                                SMS[r1 : r1 + 1, :], ONES128[:], pest_mt[:, nsl1],
                                start=first, stop=last,
                                tile_position=(0, r1),
                            )

                    # next pair's qkT chunk: elastic PE filler
                    if hp + 1 < HP and mt % 2 == 0:
                        qkt_chunk(hp + 1, mt // 2)

                    # phase 1 of the current pair
                    if hp < HP:
                        EPP = epp_pool.tile([128, 2 * N], FP16, tag="epp")
                        nc.sync.dma_start(out=EPP[:, 0:N], in_=epos_ext[h0, mt * 128 : (mt + 1) * 128, :])
                        nc.sync.dma_start(out=EPP[:, N : 2 * N], in_=epos_ext[h1, mt * 128 : (mt + 1) * 128, :])

                        SCP = ps.tile([128, 2 * N], F32, tag="sc")
                        for ns in range(2):
                            nsl = slice(ns * 512, (ns + 1) * 512)
                            nsl1 = slice(N + ns * 512, N + (ns + 1) * 512)
                            nc.tensor.matmul(
                                SCP[:, nsl],
                                QKT[0:64, hp, N + mt * 128 : N + (mt + 1) * 128],
                                QKT[0:64, hp, nsl],
                                start=True, stop=True,
                            )
                            nc.tensor.matmul(
                                SCP[:, nsl1],
                                QKT[64:128, hp, N + mt * 128 : N + (mt + 1) * 128],
                                QKT[64:128, hp, nsl],
                                start=True, stop=True,
                            )

                        ESTP = est_pool.tile([128, 2 * N], FP16, tag="est")
                        nc.scalar.activation(ESTP[:], SCP[:], EXP, scale=SCALE)
                        nc.vector.tensor_mul(ESTP[:], ESTP[:], EPP[:])
                        cur.append(ESTP)

                if prev is not None:
                    # finish pair ph: unnormalized copy, then normalize inline
                    ph = prev[0]
                    nc.scalar.copy(ATTNT[:, ph, :], OUTP[:])
                    SGP = sgp_pool.tile([97, 512], F32, tag="sgp")
                    nc.scalar.copy(SGP[0:1, :], SMS[0:1, :])
                    nc.scalar.copy(SGP[32:33, :], SMS[32:33, :])
                    nc.vector.tensor_copy(SGP[64:65, :], SMS[64:65, :])
                    nc.vector.tensor_copy(SGP[96:97, :], SMS[96:97, :])
                    # reshape sums rows to [64, 8] blocks so the iterative
                    # reciprocal runs across lanes instead of along one row
                    S2 = sgp_pool.tile([64, 32], F32, tag="s2")
                    for i, row in enumerate((0, 32, 64, 96)):
                        nc.sync.dma_start(
                            out=S2[:, 8 * i : 8 * (i + 1)],
                            in_=SGP[row : row + 1, :].rearrange(
                                "o (p f) -> o p f", p=64
                            ),
                        )
                    RI = sgp_pool.tile([64, 32], F32, tag="ri")
                    nc.vector.reciprocal(RI[:], S2[:])
                    RI16 = sgp_pool.tile([64, 32], FP16, tag="ri16")
                    nc.vector.tensor_copy(RI16[:], RI[:])
                    IR0 = invr_pool.tile([1, N], FP16, tag="invr")
                    IR1 = invr_pool.tile([1, N], FP16, tag="invr")
                    for i, (ir, ns) in enumerate(((IR0, 0), (IR0, 1), (IR1, 0), (IR1, 1))):
                        nc.sync.dma_start(
                            out=ir[0:1, 512 * ns : 512 * (ns + 1)].rearrange(
                                "o (p f) -> o p f", p=64
                            ),
                            in_=RI16[:, 8 * i : 8 * (i + 1)],
                        )
                    for ns in range(2):
                        nsl = slice(ns * 512, (ns + 1) * 512)
                        BC = ps.tile([128, 512], F32, tag="bc")
                        nc.tensor.matmul(
                            BC[0:64, :], ONES1x64[:], IR0[0:1, nsl],
                            start=True, stop=True,
                        )
                        nc.tensor.matmul(
                            BC[64:128, :], ONES1x64[:], IR1[0:1, nsl],
                            start=True, stop=True,
                            tile_position=(0, 64),
                        )
                        nc.vector.tensor_mul(
                            ATTNT[:, ph, nsl], ATTNT[:, ph, nsl], BC[:]
                        )

                if hp < HP:
                    prev = (hp, cur)

            # ---- output projection: out[n, c'] = attnT.T @ Wproj (+ b) ----
            for nt in range(NT):
                po = ps.tile([128, 1024], F32, tag=("sc", "oacc")[nt % 2])
                for cs in range(2):
                    # keep each 384-wide accumulation group inside one PSUM
                    # bank (512 fp32): place slices at 0 and 512
                    dst = po[:, cs * 512 : cs * 512 + 384]
                    for ct in range(KT):
                        nc.tensor.matmul(
                            dst,
                            ATTNT[:, ct, nt * 128 : (nt + 1) * 128],
                            WPROJ[:, ct, cs * 384 : (cs + 1) * 384],
                            start=(ct == 0),
                            stop=(ct == KT - 1 and not has_bias),
                        )
                    if has_bias:
                        nc.tensor.matmul(
                            dst,
                            ONESROW[0:1, nt * 128 : (nt + 1) * 128],
                            BPROJ[0:1, cs * 384 : (cs + 1) * 384],
                            start=False,
                            stop=True,
                        )
                OF = outsb_pool.tile([128, C], F32, tag="of")
                nc.vector.tensor_copy(
                    OF[:].rearrange("p (two x) -> p two x", two=2),
                    po[:].rearrange("p (two x) -> p two x", two=2)[:, :, 0:384],
                )
                nc.sync.dma_start(out=out_ext[nt * 128 : (nt + 1) * 128, :], in_=OF[:])

    _split_excess_waits(nc)
    return nc


_BUILT = {}


def _get_nc(has_bias):
    if has_bias not in _BUILT:
        _BUILT[has_bias] = build(has_bias)
    return _BUILT[has_bias]


def prepare_inputs(x, pos_embedding, W_qkv, b_qkv, W_proj, b_proj):
    B = x.shape[0]
    has_bias = bool(np.any(b_qkv)) or bool(np.any(b_proj))
    wqkv16 = np.ascontiguousarray(W_qkv).astype(np.float16)
    wproj16 = np.ascontiguousarray(W_proj).astype(np.float16)
    epos16 = np.exp(
        pos_embedding[0].transpose(0, 2, 1).astype(np.float32) * SCALE
    ).astype(np.float16)
    epos16 = np.ascontiguousarray(epos16)
    in_maps = []
    for b in range(B):
        m = {
            "xt": np.ascontiguousarray(x[b].T).astype(np.float16),
            "wqkv": wqkv16,
            "wproj": wproj16,
            "epos": epos16,
        }
        if has_bias:
            m["bqkv"] = b_qkv.reshape(1, -1).astype(np.float16)
            m["bproj"] = b_proj.reshape(1, -1).astype(np.float16)
        in_maps.append(m)
    return has_bias, in_maps


def kernel(x, pos_embedding, W_qkv, b_qkv, W_proj, b_proj):
    from concourse.bass_utils import run_bass_kernel_spmd

    x = np.asarray(x)
    pos_embedding = np.asarray(pos_embedding)
    W_qkv = np.asarray(W_qkv)
    b_qkv = np.asarray(b_qkv)
    W_proj = np.asarray(W_proj)
    b_proj = np.asarray(b_proj)

    has_bias, in_maps = prepare_inputs(x, pos_embedding, W_qkv, b_qkv, W_proj, b_proj)
    nc = _get_nc(has_bias)
    res = run_bass_kernel_spmd(nc, in_maps, list(range(N_CORES)), trace=False)
    out = np.stack([res.results[i]["out"] for i in range(N_CORES)], axis=0)
    return out.astype(np.float32)


# revision 24
# speedup vs baseline: 1.1628x; 1.1628x over previous
"""Multi-head self-attention with additive position bias, data-parallel across
8 TRN2 NeuronCores (one batch element per core).

Per core (batch b), everything is computed in a transposed layout so that no
on-device transposes are needed:
  - host supplies xT = x[b].T (fp16) and epos[h] = exp(pos[h].T / sqrt(D)) (fp16)
  - qT/kT    = W_{q,k}.T @ xT                    [cols, N]   (PE, fp16)
  - v        = xT.T @ W_v                        [N, cols]   (PE, fp16)
  - scoresT  = kT_h(m-tile).T @ qT_h             [m, n]      (PE, head-pairs
               packed into row groups 0-63 / 64-127 of the systolic array)
  - estT     = exp(scoresT/sqrt(D)) * eposT      (ACT exp + DVE mul; the
               additive bias becomes a multiplicative factor after exp)
  - outT_h   = v_h.T @ estT  (accumulated over m-tiles; two heads col-packed)
  - sums_h   = 1.T @ estT    (softmax denominators, M=1 matmuls col-packed)
  - normalize: outT_h * (1/sums_h) broadcast via a tiny ones-matmul
  - out      = attnT.T @ W_proj                  [N, C] fp32
"""

import numpy as np

N_CORES = 8
N = 1024
C = 768
H = 12
D = 64
HP = H // 2  # head pairs
SCALE = 0.125  # 1/sqrt(D)

# ---------------------------------------------------------------------------
# walrus in this toolchain rejects instructions carrying more than one sync
# wait ("Too many sync wait commands").  Tile's semaphore pass can attach
# several (esp. the kernel-tail drain).  Spread surplus waits across InstNoOp
# instructions inserted immediately before the oversubscribed instruction in
# the same basic block / engine stream — semantically identical, since the
# engine sequencer performs the waits in stream order.
# ---------------------------------------------------------------------------


def _apply_tile_patch():
    from concourse import mybir
    from concourse.tile import TileContext
    from concourse.vector_clock import ScopedClock

    def _patched_drain_and_barrier(self, tick_clock, wait_clock):
        nc = self.nc
        drain_inst = nc.sync.drain()
        wait_clock.add_sem_waits(
            drain_inst.ins, ScopedClock({None: tick_clock.global_clock})
        )
        nc.all_engine_barrier()
        assert self.sems is not None
        popped = nc._tile_sem_poison_stack.pop()
        assert popped is self._sem_poison
        nc.clear_and_free_semaphores(list(self.sems.allocated().values()))
        nc.all_engine_barrier()

    TileContext._drain_and_barrier = _patched_drain_and_barrier


def _split_excess_waits(nc, max_waits=1):
    from concourse import mybir

    n_split = 0
    for f in nc.m.functions:
        for blk in f.blocks:
            insts = blk.instructions
            new_list = []
            changed = False
            for inst in insts:
                si = inst.sync_info
                waits = list(si.on_wait) if (si is not None and si.on_wait) else []
                if len(waits) > max_waits:
                    extra = waits[: len(waits) - max_waits]
                    keep = waits[len(waits) - max_waits :]
                    for i in range(0, len(extra), max_waits):
                        nop = mybir.InstNoOp(
                            name=nc.get_next_instruction_name(),
                            engine=inst.engine,
                            ins=[],
                            outs=[],
                            sync_info=mybir.SyncInfo(
                                on_wait=extra[i : i + max_waits], on_update=[]
                            ),
                        )
                        nc.register_instruction(nop, overwrite=True)
                        new_list.append(nop)
                        n_split += 1
                    inst.sync_info = mybir.SyncInfo(
                        on_wait=keep,
                        on_update=list(si.on_update) if si.on_update else [],
                    )
                    changed = True
                new_list.append(inst)
            if changed:
                blk.instructions = new_list
    return n_split


def build(has_bias):
    import concourse.bass as bass
    import concourse.mybir as mybir
    from concourse.tile import TileContext

    _apply_tile_patch()

    FP16 = mybir.dt.float16
    F32 = mybir.dt.float32
    EXP = mybir.ActivationFunctionType.Exp

    nc = bass.Bass()
    xt_ext = nc.declare_dram_parameter("xt", [C, N], FP16, isOutput=False)
    wqkv_ext = nc.declare_dram_parameter("wqkv", [C, 3 * C], FP16, isOutput=False)
    wproj_ext = nc.declare_dram_parameter("wproj", [C, C], FP16, isOutput=False)
    epos_ext = nc.declare_dram_parameter("epos", [H, N, N], FP16, isOutput=False)
    if has_bias:
        bqkv_ext = nc.declare_dram_parameter("bqkv", [1, 3 * C], FP16, isOutput=False)
        bproj_ext = nc.declare_dram_parameter("bproj", [1, C], FP16, isOutput=False)
    out_ext = nc.declare_dram_parameter("out", [N, C], F32, isOutput=True)

    KT = C // 128  # 6 contraction tiles
    NT = N // 128  # 8 n-tiles / m-tiles

    with TileContext(nc) as tc:
        with (
            tc.tile_pool(name="const", bufs=1) as const,
            tc.tile_pool(name="epp", bufs=5) as epp_pool,
            tc.tile_pool(name="est", bufs=10) as est_pool,
            tc.tile_pool(name="sgp", bufs=2) as sgp_pool,
            tc.tile_pool(name="invr", bufs=3) as invr_pool,
            tc.tile_pool(name="outsb", bufs=2) as outsb_pool,
            tc.tile_pool(name="ps", bufs=1, space="PSUM") as ps,
        ):
            XT = const.tile([128, KT, N], FP16)
            WQKV = const.tile([128, KT, 3 * C], FP16)
            WPROJ = const.tile([128, KT, C], FP16)
            xt_r = xt_ext.rearrange("(t p) n -> p t n", p=128)
            wqkv_r = wqkv_ext.rearrange("(t p) n -> p t n", p=128)
            # per-k-tile pieces, v columns first: the v-projection can start
            # as soon as the first k-tile of x and Wv has landed
            for kt in range(KT):
                nc.sync.dma_start(out=XT[:, kt, :], in_=xt_r[:, kt, :])
                nc.sync.dma_start(
                    out=WQKV[:, kt, 2 * C : 3 * C], in_=wqkv_r[:, kt, 2 * C : 3 * C]
                )
            nc.sync.dma_start(out=WQKV[:, :, 0 : 2 * C], in_=wqkv_r[:, :, 0 : 2 * C])
            nc.sync.dma_start(out=WPROJ[:], in_=wproj_ext.rearrange("(t p) n -> p t n", p=128))
            if has_bias:
                BQKV = const.tile([1, 3 * C], FP16)
                BPROJ = const.tile([1, C], FP16)
                ONESROW = const.tile([1, N], FP16)
                nc.sync.dma_start(out=BQKV[:], in_=bqkv_ext[:])
                nc.sync.dma_start(out=BPROJ[:], in_=bproj_ext[:])
                nc.vector.memset(ONESROW[:], 1.0)

            ONES128 = const.tile([128, 1], FP16)
            ONES1x64 = const.tile([1, 64], FP16)
            nc.vector.memset(ONES128[:], 1.0)
            nc.vector.memset(ONES1x64[:], 1.0)


            # per pair hp: qT of heads (2hp, 2hp+1) at [0:N], kT at [N:2N]
            QKT = const.tile([128, HP, 2 * N], FP16)
            VN = const.tile([128, NT, C], FP16)
            ATTNT = const.tile([128, KT, N], FP16)

            # ---- V projection: v[n, vcol] = xT.T @ Wv (+ b_v) ----
            _vtags = ["sc", "oacc", "sums", "bc"]
            for nt in range(NT):
                for vs in range(2):
                    pv = ps.tile([128, 384], F32, tag=_vtags[(nt * 2 + vs) % 4])
                    dst = pv[:, 0:384]
                    for kt in range(KT):
                        nc.tensor.matmul(
                            dst,
                            XT[:, kt, nt * 128 : (nt + 1) * 128],
                            WQKV[:, kt, 2 * C + vs * 384 : 2 * C + (vs + 1) * 384],
                            start=(kt == 0),
                            stop=(kt == KT - 1 and not has_bias),
                        )
                    if has_bias:
                        nc.tensor.matmul(
                            dst,
                            ONESROW[0:1, nt * 128 : (nt + 1) * 128],
                            BQKV[0:1, 2 * C + vs * 384 : 2 * C + (vs + 1) * 384],
                            start=False,
                            stop=True,
                        )
                    nc.vector.tensor_copy(VN[:, nt, vs * 384 : (vs + 1) * 384], dst)

            # ---- head-pair loop, software-pipelined one pair deep:
            # pair hp:   scores -> exp -> est     (ACT-bound phase)
            # pair hp-1: attn.v + sums matmuls    (dense PE work, fills gaps)
            # pair hp+1: qT/kT projection chunks  (always-ready PE filler that
            #            keeps the HAM activity window busy -> PE stays warm)

            def qkt_chunk(pair, c):
                # chunk c of pair: c in 0..3 -> (q ns0, q ns1, k ns0, k ns1)
                ct = pair if c < 2 else HP + pair
                col0 = ct * 128
                ns = c % 2
                pqc = ps.tile([128, 512], F32, tag="bc")
                for kt in range(KT):
                    nc.tensor.matmul(
                        pqc[:],
                        WQKV[:, kt, col0 : col0 + 128],
                        XT[:, kt, ns * 512 : (ns + 1) * 512],
                        start=(kt == 0),
                        stop=(kt == KT - 1 and not has_bias),
                    )
                if has_bias:
                    nc.tensor.matmul(
                        pqc[:],
                        BQKV[0:1, col0 : col0 + 128],
                        ONESROW[0:1, ns * 512 : (ns + 1) * 512],
                        start=False,
                        stop=True,
                    )
                nc.vector.tensor_copy(QKT[:, pair, c * 512 : (c + 1) * 512], pqc[:])

            for c in range(4):
                qkt_chunk(0, c)

            prev = None  # (hp, [EST per mt])
            for hp in range(HP + 1):
                if hp < HP:
                    h0, h1 = 2 * hp, 2 * hp + 1
                cur = []
                OUTP = SMS = None
                ph0 = ph1 = None
                if prev is not None:
                    ph, pest = prev
                    ph0, ph1 = 2 * ph, 2 * ph + 1
                for mt in range(8):
                    # phase 2 of the previous pair first: its inputs are all
                    # ready, so the PE never stalls entering an iteration
                    if prev is not None:
                        pest_mt = prev[1][mt]
                        if mt == 0:
                            OUTP = ps.tile([128, 1024], F32, tag="oacc")
                            # one PSUM bank: rows 0/32 = h0 (ns0/ns1), 64/96 = h1
                            SMS = ps.tile([97, 512], F32, tag="sums")
                        first, last = (mt == 0), (mt == 7)
                        for ns in range(2):
                            nsl = slice(ns * 512, (ns + 1) * 512)
                            nsl1 = slice(N + ns * 512, N + (ns + 1) * 512)
                            nc.tensor.matmul(
                                OUTP[0:64, nsl],
                                VN[:, mt, ph0 * D : (ph0 + 1) * D],
                                pest_mt[:, nsl],
                                start=first, stop=last,
                            )
                            nc.tensor.matmul(
                                OUTP[64:128, nsl],
                                VN[:, mt, ph1 * D : (ph1 + 1) * D],
                                pest_mt[:, nsl1],
                                start=first, stop=last,
                                tile_position=(0, 64),
                            )
                        for ns in range(2):
                            nsl = slice(ns * 512, (ns + 1) * 512)
                            nsl1 = slice(N + ns * 512, N + (ns + 1) * 512)
                            r0, r1 = 32 * ns, 64 + 32 * ns
                            nc.tensor.matmul(
                                SMS[r0 : r0 + 1, :], ONES128[:], pest_mt[:, nsl],
                                start=first, stop=last,
                                tile_position=(0, r0),
                            )
                            nc.tensor.matmul(
                                SMS[r1 : r1 + 1, :], ONES128[:], pest_mt[:, nsl1],
                                start=first, stop=last,
                                tile_position=(0, r1),
                            )

                    # next pair's qkT chunk: elastic PE filler
                    if hp + 1 < HP and mt % 2 == 0:
                        qkt_chunk(hp + 1, mt // 2)

                    # phase 1 of the current pair
                    if hp < HP:
                        EPP = epp_pool.tile([128, 2 * N], FP16, tag="epp")
                        nc.sync.dma_start(out=EPP[:, 0:N], in_=epos_ext[h0, mt * 128 : (mt + 1) * 128, :])
                        nc.sync.dma_start(out=EPP[:, N : 2 * N], in_=epos_ext[h1, mt * 128 : (mt + 1) * 128, :])

                        SCP = ps.tile([128, 2 * N], F32, tag="sc")
                        for ns in range(2):
                            nsl = slice(ns * 512, (ns + 1) * 512)
                            nsl1 = slice(N + ns * 512, N + (ns + 1) * 512)
                            nc.tensor.matmul(
                                SCP[:, nsl],
                                QKT[0:64, hp, N + mt * 128 : N + (mt + 1) * 128],
                                QKT[0:64, hp, nsl],
                                start=True, stop=True,
                            )
                            nc.tensor.matmul(
                                SCP[:, nsl1],
                                QKT[64:128, hp, N + mt * 128 : N + (mt + 1) * 128],
                                QKT[64:128, hp, nsl],
                                start=True, stop=True,
                            )

                        ESTP = est_pool.tile([128, 2 * N], FP16, tag="est")
                        nc.scalar.activation(ESTP[:], SCP[:], EXP, scale=SCALE)
                        nc.vector.tensor_mul(ESTP[:], ESTP[:], EPP[:])
                        cur.append(ESTP)

                if prev is not None:
                    # finish pair ph: unnormalized copy, then normalize inline
                    ph = prev[0]
                    nc.scalar.copy(ATTNT[:, ph, :], OUTP[:])
                    SGP = sgp_pool.tile([97, 512], F32, tag="sgp")
                    nc.scalar.copy(SGP[0:1, :], SMS[0:1, :])
                    nc.scalar.copy(SGP[32:33, :], SMS[32:33, :])
                    nc.vector.tensor_copy(SGP[64:65, :], SMS[64:65, :])
                    nc.vector.tensor_copy(SGP[96:97, :], SMS[96:97, :])
                    # reshape sums rows to [64, 8] blocks so the iterative
                    # reciprocal runs across lanes instead of along one row
                    S2 = sgp_pool.tile([64, 32], F32, tag="s2")
                    for i, row in enumerate((0, 32, 64, 96)):
                        nc.sync.dma_start(
                            out=S2[:, 8 * i : 8 * (i + 1)],
                            in_=SGP[row : row + 1, :].rearrange(
                                "o (p f) -> o p f", p=64
                            ),
                        )
                    RI = sgp_pool.tile([64, 32], F32, tag="ri")
                    nc.vector.reciprocal(RI[:], S2[:])
                    RI16 = sgp_pool.tile([64, 32], FP16, tag="ri16")
                    nc.vector.tensor_copy(RI16[:], RI[:])
                    IR0 = invr_pool.tile([1, N], FP16, tag="invr")
                    IR1 = invr_pool.tile([1, N], FP16, tag="invr")
                    for i, (ir, ns) in enumerate(((IR0, 0), (IR0, 1), (IR1, 0), (IR1, 1))):
                        nc.sync.dma_start(
                            out=ir[0:1, 512 * ns : 512 * (ns + 1)].rearrange(
                                "o (p f) -> o p f", p=64
                            ),
                            in_=RI16[:, 8 * i : 8 * (i + 1)],
                        )
                    for ns in range(2):
                        nsl = slice(ns * 512, (ns + 1) * 512)
                        BC = ps.tile([128, 512], F32, tag="bc")
                        nc.tensor.matmul(
                            BC[0:64, :], ONES1x64[:], IR0[0:1, nsl],
                            start=True, stop=True,
                        )
                        nc.tensor.matmul(
                            BC[64:128, :], ONES1x64[:], IR1[0:1, nsl],
                            start=True, stop=True,
                            tile_position=(0, 64),
                        )
                        nc.vector.tensor_mul(
                            ATTNT[:, ph, nsl], ATTNT[:, ph, nsl], BC[:]
                        )

                if hp < HP:
                    prev = (hp, cur)

            # ---- output projection: out[n, c'] = attnT.T @ Wproj (+ b) ----
            for nt in range(NT):
                po = ps.tile([128, 1024], F32, tag=("sc", "oacc")[nt % 2])
                for cs in range(2):
                    # keep each 384-wide accumulation group inside one PSUM
                    # bank (512 fp32): place slices at 0 and 512
                    dst = po[:, cs * 512 : cs * 512 + 384]
                    for ct in range(KT):
                        nc.tensor.matmul(
                            dst,
                            ATTNT[:, ct, nt * 128 : (nt + 1) * 128],
                            WPROJ[:, ct, cs * 384 : (cs + 1) * 384],
                            start=(ct == 0),
                            stop=(ct == KT - 1 and not has_bias),
                        )
                    if has_bias:
                        nc.tensor.matmul(
                            dst,
                            ONESROW[0:1, nt * 128 : (nt + 1) * 128],
                            BPROJ[0:1, cs * 384 : (cs + 1) * 384],
                            start=False,
                            stop=True,
                        )
                OF = outsb_pool.tile([128, C], F32, tag="of")
                nc.vector.tensor_copy(
                    OF[:].rearrange("p (two x) -> p two x", two=2),
                    po[:].rearrange("p (two x) -> p two x", two=2)[:, :, 0:384],
                )
                nc.sync.dma_start(out=out_ext[nt * 128 : (nt + 1) * 128, :], in_=OF[:])

    _split_excess_waits(nc)
    return nc


_BUILT = {}


def _get_nc(has_bias):
    if has_bias not in _BUILT:
        _BUILT[has_bias] = build(has_bias)
    return _BUILT[has_bias]


def prepare_inputs(x, pos_embedding, W_qkv, b_qkv, W_proj, b_proj):
    B = x.shape[0]
    has_bias = bool(np.any(b_qkv)) or bool(np.any(b_proj))
    wqkv16 = np.ascontiguousarray(W_qkv).astype(np.float16)
    wproj16 = np.ascontiguousarray(W_proj).astype(np.float16)
    epos16 = np.exp(
        pos_embedding[0].transpose(0, 2, 1).astype(np.float32) * SCALE
    ).astype(np.float16)
    epos16 = np.ascontiguousarray(epos16)
    in_maps = []
    for b in range(B):
        m = {
            "xt": np.ascontiguousarray(x[b].T).astype(np.float16),
            "wqkv": wqkv16,
            "wproj": wproj16,
            "epos": epos16,
        }
        if has_bias:
            m["bqkv"] = b_qkv.reshape(1, -1).astype(np.float16)
            m["bproj"] = b_proj.reshape(1, -1).astype(np.float16)
        in_maps.append(m)
    return has_bias, in_maps


def kernel(x, pos_embedding, W_qkv, b_qkv, W_proj, b_proj):
    from concourse.bass_utils import run_bass_kernel_spmd

    x = np.asarray(x)
    pos_embedding = np.asarray(pos_embedding)
    W_qkv = np.asarray(W_qkv)
    b_qkv = np.asarray(b_qkv)
    W_proj = np.asarray(W_proj)
    b_proj = np.asarray(b_proj)

    has_bias, in_maps = prepare_inputs(x, pos_embedding, W_qkv, b_qkv, W_proj, b_proj)
    nc = _get_nc(has_bias)
    res = run_bass_kernel_spmd(nc, in_maps, list(range(N_CORES)), trace=False)
    out = np.stack([res.results[i]["out"] for i in range(N_CORES)], axis=0)
    return out.astype(np.float32)
